# revision 53
# baseline (speedup 1.0000x reference)
"""Trainium2 Bass kernel for a single causal attention head (with the
faithful source bug: q = x @ W_key, W_query unused).

Full-input contract: kernel(x, W_key, W_query, W_value) -> [8, 2048, 128].
Sharding: data-parallel over batch B=8 across 8 NeuronCores (1 batch/core).

Per-core math (T=2048, C=1024, H=128):
    K = x @ W_key            (V = x @ W_value)
    S = K @ K.T * H**-0.5    (symmetric since q == k)
    out = softmax(causal(S)) @ V

v5 design (profile-driven; the ACT engine's exp stream is the
mid-phase critical path, so everything is organized to start it as
early as possible and keep it gapless):
  - Projections split by T-halves: K for queries/keys [0,1024) only
    needs the first half of the x DMA, so scores+exp for the top-left
    triangle start ~4us after half the input landed, overlapping the
    rest of the DMA and the second projection half. Phase A = rows
    0-7 queries [128j,1024); phase B = rows 0-7 queries [1024,2048)
    then rows 8-15.
  - exp in <=1024-wide PSUM pieces (24 ACTIVATEs), diag tiles exp'd
    unmasked and masked post-hoc in fp16 (off the PE->ACT chain).
  - AV per (j, i): stationary E_ji [k, q in tile i], moving
    vaug_j [k, v|ones] — denominators ride for free; back-to-back AV
    matmuls sustain ~57ns (LDWEIGHTS pipelined), so AV is cheap.
    Columns pack three per PSUM bank (640B regions, pre-zeroed,
    start=False accumulation) so 12+ columns are concurrently open
    and the work spreads across all rounds instead of piling after
    the last exp.
  - V^T -> per-tile V [t, h] via per-half XBAR DMA transposes.
  - Outputs batch 4 seq tiles per DMA. Input DMA triggers balanced
    across the two HWDGE queues so xT chunk 0 is never queued behind
    the weights.
"""

import numpy as np

import concourse.bass as bass
import concourse.mybir as mybir
import concourse.tile as tile
from concourse import bacc, bass_utils


P = 128
T = 2048
C = 1024
H = 128
NT = T // P  # 16 seq tiles
NC = C // P  # 8 contraction tiles
NCORES = 8
NAV = P + 1  # v | ones
HT = T // 2  # half of t
SCALE = float(H) ** -0.5
F32 = mybir.dt.float32
FP16 = mybir.dt.float16
EXP = mybir.ActivationFunctionType.Exp

CHW = 512
NWARM = 2
NR = 25  # rounds: R0-7 phase A, R8-23 phase B, R24 final drains
REG = 160  # f32 region stride inside an AV bank (3 x 129-wide columns)


# AV column -> rounds window. Columns pack 3/bank; banks become free
# progressively (they reuse projection-psum slots after the casts):
# bank0/1 (cols 0-5) at R5, bank2 (cols 6-8) at R6(+1), bank3
# (cols 9-11) at R7(+2); gen-1: cols 12-14 on bank0 after R9 re-zero,
# col 15 on bank1 after R10. Cols >= 8 also need phase-B pieces:
# update (j, i) valid at R >= j+8.
AV_ROUNDS = {
    0: [5], 1: [5, 6], 2: [5, 6], 3: [5, 6, 7], 4: [5, 6, 7],
    5: [5, 6, 7], 6: [9, 10], 7: [9, 10, 11],
    8: list(range(9, 17)), 9: list(range(10, 18)),
    10: list(range(10, 19)), 11: list(range(10, 20)),
    12: list(range(14, 21)), 13: list(range(14, 22)),
    14: list(range(14, 23)), 15: list(range(14, 24)),
}


def build_av_schedule():
    sched = {}
    for i in range(NT):
        rounds = AV_ROUNDS[i]
        js = list(range(i + 1))
        k = len(rounds)
        base, rem = divmod(len(js), k)
        sizes = [base + (1 if x < rem else 0) for x in range(k)]
        pos = 0
        for r, sz in zip(rounds, sizes):
            for j in js[pos : pos + sz]:
                if i >= 8:
                    assert j <= r - 8, (i, j, r)
                sched.setdefault(r, []).append((j, i))
            pos += sz
    return sched


AV_SCHED = build_av_schedule()
# drain rounds (column close = last round of AV_ROUNDS, +1, 2/round)
DRAINS = {
    7: [0, 1], 8: [2, 3], 9: [4, 5], 11: [6], 12: [7],
    17: [8], 18: [9], 19: [10], 20: [11],
    21: [12], 22: [13], 23: [14], 24: [15],
}
YDMA = {8: 0, 12: 1, 20: 2, 24: 3}  # round -> y group
# bank -> columns generations: b0: (0,1,2)->(6,7,8); b1: (3,4,5)->
# (9,10,11); b2: (12,13,14); b3: (15,)
AV_BANK = {}
for _i in range(NT):
    AV_BANK[_i] = (
        (_i // 3, _i % 3) if _i <= 5 else
        ((_i - 6) // 3, (_i - 6) % 3) if _i <= 11 else
        (2, _i - 12) if _i <= 14 else (3, 0)
    )


def build_module():
    nc = bacc.Bacc(
        "TRN2", target_bir_lowering=False, debug=False, num_devices=NCORES
    )
    # t-halves pre-split on host so each half-chunk is a contiguous
    # 256KB DRAM blob (best DMA descriptor efficiency)
    xT_d = nc.dram_tensor("xT", [2, C, HT], FP16, kind="ExternalInput").ap()
    wk_d = nc.dram_tensor("WK", [P, NC, H], FP16, kind="ExternalInput").ap()
    wv_d = nc.dram_tensor("WV", [P, NC, H], FP16, kind="ExternalInput").ap()
    cons_d = nc.dram_tensor("CONS", [P, 2, P], FP16, kind="ExternalInput").ap()
    y_d = nc.dram_tensor("y", [T, H], F32, kind="ExternalOutput").ap()

    offs = []
    off = 0
    for j in range(NT):
        offs.append(off)
        off += (NT - j) * P
    e_width = off  # 17408

    def rw(j):
        return (NT - j) * P

    with tile.TileContext(nc) as tc:
        with (
            tc.tile_pool(name="const", bufs=1) as const,
            tc.tile_pool(name="xt", bufs=1) as xt_pool,
            tc.tile_pool(name="kv", bufs=1) as kv,
            tc.tile_pool(name="e", bufs=1) as e_pool,
            tc.tile_pool(name="ysb", bufs=2) as ysb_pool,
            tc.tile_pool(name="rcp", bufs=4) as rcp_pool,
        ):
            wk_sb = const.tile([P, NC, H], FP16)
            wv_sb = const.tile([P, NC, H], FP16)
            cons = const.tile([P, 2, P], FP16)

            # x half-chunks [c-block, t-half], one DMA each (pairing
            # them fragments the descriptors into 512B packets). WK and
            # the half-0 chunks land first so K-half-0 closes ~16.2us;
            # WV mid-stream before the V-half-0 matmuls need it.
            xt_all = xt_pool.tile([P, NC, T], FP16)

            def xt(c):
                return xt_all[:, c, :]

            def xchunk(c, h):
                lo = h * HT
                # three-way queue split: the two HWDGE queues sustain
                # only ~120GB/s each; GpSimd's software DGE adds a third
                eng = (
                    nc.gpsimd if c in (2, 5)
                    else nc.sync if c % 2 == 0 else nc.scalar
                )
                eng.dma_start(
                    xt_all[:, c, lo : lo + HT],
                    xT_d[h, c * P : (c + 1) * P, :],
                )

            nc.sync.dma_start(cons[:], cons_d[:])
            nc.scalar.dma_start(wk_sb[:], wk_d[:])
            for c in range(NC):
                xchunk(c, 0)
            nc.sync.dma_start(wv_sb[:], wv_d[:])
            for c in range(NC):
                xchunk(c ^ 1, 1)  # flip queues to balance bytes
            umask = cons[:, 0]

            warm = const.tile([P, 1], F32)
            nc.vector.memset(warm[:], 0.0)
            nc.scalar.activation(warm[:], warm[:], EXP)

            kt_r = kv.tile([P, T], FP16)  # K^T [h, t]
            vt_sb = kv.tile([P, T], FP16)  # V^T [h, t]
            vtmp = kv.tile([P, NT, P], FP16)  # XBAR dst: V [t, h] tiles
            vaug = kv.tile([P, NT, NAV], FP16)  # [v | ones]
            nc.vector.memset(vaug[:], 1.0)
            e_all = e_pool.tile([P, e_width], FP16)

            with (
                tc.tile_pool(name="psproj", bufs=4, space="PSUM") as psp,
                tc.tile_pool(name="pssc", bufs=2, space="PSUM") as pssc,
            ):
                # --- static PSUM plan: 4 proj/AV banks + 4 score banks.
                # Half-1 projection tiles reuse half-0's slots (freed by
                # the casts), and the 4 AV banks reuse the proj slots
                # after the half-1 casts. Warm-ups write into the first
                # kt half-0 bank before its real start=True matmul.
                trash = const.tile([P, CHW], FP16)
                nc.vector.memset(trash[:], 0.0)
                proj_ps = {}
                for h in range(2):
                    for kvi in range(2):
                        proj_ps[(h, kvi)] = [
                            psp.tile(
                                [P, CHW], F32, tag="ps", name=f"pj{h}{kvi}{ch}"
                            )
                            for ch in range(2)
                        ]

                for _ in range(NWARM):
                    nc.tensor.matmul(
                        proj_ps[(0, 0)][0][:], trash[:, 0:P], trash[:],
                        start=True, stop=True, skip_group_check=True,
                    )

                def proj_mms(h, kvi, cs):
                    w_src = wk_sb if kvi == 0 else wv_sb
                    for c in cs:
                        for ch in range(2):
                            lo = h * HT + ch * CHW
                            nc.tensor.matmul(
                                proj_ps[(h, kvi)][ch][:],
                                w_src[:, c, :],
                                xt(c)[:, lo : lo + CHW],
                                start=(c == 0),
                                stop=(c == NC - 1),
                            )

                def cast_half(h, kvi, engines):
                    dst = kt_r if kvi == 0 else vt_sb
                    for ch in range(2):
                        lo = h * HT + ch * CHW
                        eng = engines[ch]
                        if eng is nc.scalar:
                            nc.scalar.copy(
                                dst[:, lo : lo + CHW], proj_ps[(h, kvi)][ch][:]
                            )
                        else:
                            nc.vector.tensor_copy(
                                dst[:, lo : lo + CHW], proj_ps[(h, kvi)][ch][:]
                            )

                # --- AV banks: 3 columns per bank, memset + start=False
                # accumulation; allocated progressively as the proj
                # slots they reuse are freed by the casts.
                av_banks = {}

                def av_bank_open(b):
                    av_banks[b] = psp.tile(
                        [P, 512], F32, tag="ps", name=f"avb{b}"
                    )
                    nc.vector.memset(av_banks[b][:], 0.0)

                def av_region(i):
                    b, reg = AV_BANK[i]
                    return av_banks[b][:, REG * reg : REG * reg + NAV]

                # --- scores / exp pieces ------------------------------
                def spiece(j, part):
                    """part 0: A piece (queries [128j, 1024) for j<8,
                    whole row for j>=8); part 1: B piece [1024, 2048)."""
                    if j < 8:
                        wA = 1024 - j * P
                        base, wt = (0, wA) if part == 0 else (wA, 1024)
                    else:
                        base, wt = 0, rw(j)
                    b0 = j * P
                    s_ps = pssc.tile(
                        [P, 1024], F32, tag="ps", name=f"sp{j}_{part}"
                    )
                    p2 = 0
                    while p2 < wt:
                        w2 = min(CHW, wt - p2)
                        nc.tensor.matmul(
                            s_ps[:, p2 : p2 + w2],
                            kt_r[:, b0 : b0 + P],
                            kt_r[:, b0 + base + p2 : b0 + base + p2 + w2],
                            start=True,
                            stop=True,
                        )
                        p2 += w2
                    nc.scalar.activation(
                        e_all[:, offs[j] + base : offs[j] + base + wt],
                        s_ps[:, :wt],
                        EXP,
                        scale=SCALE,
                    )

                def mask_row(j):
                    nc.vector.tensor_mul(
                        e_all[:, offs[j] : offs[j] + P],
                        e_all[:, offs[j] : offs[j] + P],
                        umask[:],
                    )

                def av_update(j, i):
                    eji = e_all[
                        :, offs[j] + (i - j) * P : offs[j] + (i - j + 1) * P
                    ]
                    nc.tensor.matmul(
                        av_region(i),
                        eji,
                        vaug[:, j, :],
                        start=False,
                        stop=(j == i),
                        skip_group_check=True,
                    )

                y_tiles = {}

                def drain_col(i):
                    g = i // 4
                    if i % 4 == 0:
                        y_tiles[g] = ysb_pool.tile(
                            [P, 4, P], F32, tag="ysb", name=f"ysb{g}"
                        )
                    av = av_region(i)
                    recip = rcp_pool.tile(
                        [P, 1], F32, tag="recip", name=f"rcp{i}"
                    )
                    nc.vector.reciprocal(recip[:], av[:, P : P + 1])
                    nc.vector.tensor_scalar_mul(
                        y_tiles[g][:, i % 4, :], av[:, 0:P], recip[:]
                    )

                def y_out(g):
                    y_view = y_d[512 * g : 512 * (g + 1), :].rearrange(
                        "(i p) h -> p i h", p=P
                    )
                    nc.sync.dma_start(y_view, y_tiles[g][:])

                vaug_done = [0]

                def vaug_copy_upto(n):
                    while vaug_done[0] < min(n, NT):
                        j = vaug_done[0]
                        nc.vector.tensor_copy(
                            vaug[:, j, 0:P], vtmp[:, j, :]
                        )
                        vaug_done[0] += 1

                # --- pre-round emission -------------------------------
                proj_mms(0, 0, range(NC))  # K half-0
                cast_half(0, 0, (nc.scalar, nc.vector))  # kt h0
                spiece(0, 0)  # scores row 0 piece A -> first exp

                # --- round loop ---------------------------------------
                for r in range(NR):
                    for i in DRAINS.get(r, ()):
                        drain_col(i)
                    # masks: rows 0-7 at R0-7, rows 8-15 at R16-23
                    if r < 8:
                        mask_row(r)
                    elif 16 <= r < 24:
                        mask_row(r - 8)
                    # scores pieces: A rows 1-7 at R0-6; B row rb at
                    # R7+rb; rows 8-15 at R15-22
                    if r < 7:
                        spiece(r + 1, 0)
                    elif r < 15:
                        spiece(r - 7, 1)
                    elif r < 23:
                        spiece(r - 7, 0)
                    # V half-0 early (weights land with the half-0 DMA)
                    if r == 0:
                        proj_mms(0, 1, range(0, 4))
                    if r == 1:
                        proj_mms(0, 1, range(4, NC))
                        cast_half(0, 1, (nc.vector, nc.vector))
                        nc.sync.dma_start_transpose(
                            vtmp[:, 0:8, :], vt_sb[:, 0:HT]
                        )
                    # K half-1 as its chunks land; casts feed phase B
                    if 1 <= r <= 4:
                        proj_mms(1, 0, range(2 * (r - 1), 2 * r))
                    if r == 4:
                        cast_half(1, 0, (nc.scalar, nc.vector))
                    if r == 5:
                        av_bank_open(0)  # reuses kt_h1 slots (cast @R4)
                        av_bank_open(1)
                    # V half-1 dribbled over R8-13 (only gates AV js>=8,
                    # first used R16) so the B score pieces aren't
                    # starved behind its matmuls
                    if 8 <= r <= 9:
                        proj_mms(1, 1, range(2 * (r - 8), 2 * (r - 7)))
                    if 10 <= r <= 13:
                        proj_mms(1, 1, [r - 6])
                    if r == 13:
                        cast_half(1, 1, (nc.vector, nc.vector))
                        nc.sync.dma_start_transpose(
                            vtmp[:, 8:16, :], vt_sb[:, HT:T]
                        )
                    if r == 14:
                        av_bank_open(2)  # reuse vt_h1 slots (cast @R13)
                        av_bank_open(3)
                    # inter-generation AV bank re-zeros
                    if r == 9:
                        nc.vector.memset(av_banks[0][:], 0.0)
                    if r == 10:
                        nc.vector.memset(av_banks[1][:], 0.0)
                    # vaug tiles: 0-7 after XBAR-A, 8-15 after XBAR-B
                    # (which is only emitted at R13)
                    if r >= 2:
                        vaug_copy_upto(8 if r < 14 else 8 + 8 * (r - 13))
                    for j, i in AV_SCHED.get(r, ()):
                        av_update(j, i)
                    if r in YDMA:
                        y_out(YDMA[r])

    nc.compile()
    return nc


_NC_CACHE = None


def _get_module():
    global _NC_CACHE
    if _NC_CACHE is None:
        _NC_CACHE = build_module()
    return _NC_CACHE


def run(in_maps, trace=False, **kw):
    nc = _get_module()
    return bass_utils.run_bass_kernel_spmd(
        nc, in_maps, core_ids=list(range(NCORES)), trace=trace, **kw
    )


def make_in_maps(x, W_key, W_value):
    x = np.asarray(x, dtype=np.float32).astype(np.float16)
    xT = x.transpose(0, 2, 1)  # [B, C, T]
    # [B, 2, C, HT]: t-halves contiguous
    xT = np.ascontiguousarray(
        xT.reshape(NCORES, C, 2, HT).transpose(0, 2, 1, 3)
    )
    wk = np.asarray(W_key, np.float32).astype(np.float16)
    wk = np.ascontiguousarray(wk.reshape(NC, P, H).transpose(1, 0, 2))
    wv = np.asarray(W_value, np.float32).astype(np.float16)
    wv = np.ascontiguousarray(wv.reshape(NC, P, H).transpose(1, 0, 2))
    umask = np.triu(np.ones((P, P), dtype=np.float16))  # keep q >= k
    ident = np.eye(P, dtype=np.float16)
    cons = np.ascontiguousarray(np.stack([umask, ident], axis=1))
    return [
        {"xT": xT[b], "WK": wk, "WV": wv, "CONS": cons}
        for b in range(NCORES)
    ]


def kernel(x, W_key, W_query, W_value):
    # W_query intentionally unused: the reference applies W_key for q too.
    del W_query
    res = run(make_in_maps(x, W_key, W_value), trace=False)
    return np.stack([res.results[b]["y"] for b in range(NCORES)], axis=0)


# revision 54
# speedup vs baseline: 1.1782x; 1.1782x over previous
"""Trainium2 Bass kernel for a single causal attention head (with the
faithful source bug: q = x @ W_key, W_query unused).

Full-input contract: kernel(x, W_key, W_query, W_value) -> [8, 2048, 128].
Sharding: data-parallel over batch B=8 across 8 NeuronCores (1 batch/core).

Per-core math (T=2048, C=1024, H=128):
    K = x @ W_key            (V = x @ W_value)
    S = K @ K.T * H**-0.5    (symmetric since q == k)
    out = softmax(causal(S)) @ V

v5 design (profile-driven; the ACT engine's exp stream is the
mid-phase critical path, so everything is organized to start it as
early as possible and keep it gapless):
  - Projections split by T-halves: K for queries/keys [0,1024) only
    needs the first half of the x DMA, so scores+exp for the top-left
    triangle start ~4us after half the input landed, overlapping the
    rest of the DMA and the second projection half. Phase A = rows
    0-7 queries [128j,1024); phase B = rows 0-7 queries [1024,2048)
    then rows 8-15.
  - exp in <=1024-wide PSUM pieces (24 ACTIVATEs), diag tiles exp'd
    unmasked and masked post-hoc in fp16 (off the PE->ACT chain).
  - AV per (j, i): stationary E_ji [k, q in tile i], moving
    vaug_j [k, v|ones] — denominators ride for free; back-to-back AV
    matmuls sustain ~57ns (LDWEIGHTS pipelined), so AV is cheap.
    Columns pack three per PSUM bank (640B regions, pre-zeroed,
    start=False accumulation) so 12+ columns are concurrently open
    and the work spreads across all rounds instead of piling after
    the last exp.
  - V^T -> per-tile V [t, h] via per-half XBAR DMA transposes.
  - Outputs batch 4 seq tiles per DMA. Input DMA triggers balanced
    across the two HWDGE queues so xT chunk 0 is never queued behind
    the weights.
"""

import numpy as np

import concourse.bass as bass
import concourse.mybir as mybir
import concourse.tile as tile
from concourse import bacc, bass_utils


P = 128
T = 2048
C = 1024
H = 128
NT = T // P  # 16 seq tiles
NC = C // P  # 8 contraction tiles
NCORES = 8
NAV = P + 1  # v | ones
HT = T // 2  # half of t
SCALE = float(H) ** -0.5
F32 = mybir.dt.float32
FP16 = mybir.dt.float16
EXP = mybir.ActivationFunctionType.Exp

CHW = 512
NWARM = 2
NR = 25  # rounds: R0-7 phase A, R8-23 phase B, R24 final drains
REG = 160  # f32 region stride inside an AV bank (3 x 129-wide columns)


# AV column -> rounds window. Columns pack 3/bank; banks become free
# progressively (they reuse projection-psum slots after the casts):
# bank0/1 (cols 0-5) at R5, bank2 (cols 6-8) at R6(+1), bank3
# (cols 9-11) at R7(+2); gen-1: cols 12-14 on bank0 after R9 re-zero,
# col 15 on bank1 after R10. Cols >= 8 also need phase-B pieces:
# update (j, i) valid at R >= j+8.
AV_ROUNDS = {
    0: [5], 1: [5, 6], 2: [5, 6], 3: [5, 6, 7], 4: [5, 6, 7],
    5: [5, 6, 7], 6: [9, 10], 7: [9, 10, 11],
    8: list(range(9, 17)), 9: list(range(10, 18)),
    10: list(range(10, 19)), 11: list(range(10, 20)),
    12: list(range(14, 21)), 13: list(range(14, 22)),
    14: list(range(14, 23)), 15: list(range(14, 24)),
}


def build_av_schedule():
    sched = {}
    for i in range(NT):
        rounds = AV_ROUNDS[i]
        js = list(range(i + 1))
        k = len(rounds)
        base, rem = divmod(len(js), k)
        sizes = [base + (1 if x < rem else 0) for x in range(k)]
        pos = 0
        for r, sz in zip(rounds, sizes):
            for j in js[pos : pos + sz]:
                if i >= 8:
                    assert j <= r - 8, (i, j, r)
                sched.setdefault(r, []).append((j, i))
            pos += sz
    return sched


AV_SCHED = build_av_schedule()
# drain rounds (column close = last round of AV_ROUNDS, +1, 2/round)
DRAINS = {
    7: [0, 1], 8: [2, 3], 9: [4, 5], 11: [6], 12: [7],
    17: [8], 18: [9], 19: [10], 20: [11],
    21: [12], 22: [13], 23: [14], 24: [15],
}
YDMA = {8: 0, 12: 1, 20: 2, 24: 3}  # round -> y group
# bank -> columns generations: b0: (0,1,2)->(6,7,8); b1: (3,4,5)->
# (9,10,11); b2: (12,13,14); b3: (15,)
AV_BANK = {}
for _i in range(NT):
    AV_BANK[_i] = (
        (_i // 3, _i % 3) if _i <= 5 else
        ((_i - 6) // 3, (_i - 6) % 3) if _i <= 11 else
        (2, _i - 12) if _i <= 14 else (3, 0)
    )


def build_module():
    nc = bacc.Bacc(
        "TRN2", target_bir_lowering=False, debug=False, num_devices=NCORES
    )
    # t-halves pre-split on host so each half-chunk is a contiguous
    # 256KB DRAM blob (best DMA descriptor efficiency)
    xT_d = nc.dram_tensor("xT", [2, C, HT], FP16, kind="ExternalInput").ap()
    wk_d = nc.dram_tensor("WK", [P, NC, H], FP16, kind="ExternalInput").ap()
    wv_d = nc.dram_tensor("WV", [P, NC, H], FP16, kind="ExternalInput").ap()
    cons_d = nc.dram_tensor("CONS", [P, 2, P], FP16, kind="ExternalInput").ap()
    y_d = nc.dram_tensor("y", [T, H], F32, kind="ExternalOutput").ap()

    offs = []
    off = 0
    for j in range(NT):
        offs.append(off)
        off += (NT - j) * P
    e_width = off  # 17408

    def rw(j):
        return (NT - j) * P

    with tile.TileContext(nc) as tc:
        with (
            tc.tile_pool(name="const", bufs=1) as const,
            tc.tile_pool(name="xt", bufs=1) as xt_pool,
            tc.tile_pool(name="kv", bufs=1) as kv,
            tc.tile_pool(name="e", bufs=1) as e_pool,
            tc.tile_pool(name="ysb", bufs=2) as ysb_pool,
            tc.tile_pool(name="rcp", bufs=4) as rcp_pool,
        ):
            wk_sb = const.tile([P, NC, H], FP16)
            wv_sb = const.tile([P, NC, H], FP16)
            cons = const.tile([P, 2, P], FP16)

            # x half-chunks [c-block, t-half], one DMA each (pairing
            # them fragments the descriptors into 512B packets). WK and
            # the half-0 chunks land first so K-half-0 closes ~16.2us;
            # WV mid-stream before the V-half-0 matmuls need it.
            xt_all = xt_pool.tile([P, NC, T], FP16)

            def xt(c):
                return xt_all[:, c, :]

            def xchunk(c, h):
                lo = h * HT
                # three-way queue split: each queue sustains ~120GB/s,
                # so balance bytes evenly (GpSimd SWDGE is the third)
                eng = (nc.sync, nc.scalar, nc.gpsimd)[c % 3]
                eng.dma_start(
                    xt_all[:, c, lo : lo + HT],
                    xT_d[h, c * P : (c + 1) * P, :],
                )

            nc.sync.dma_start(cons[:], cons_d[:])
            nc.gpsimd.dma_start(wk_sb[:], wk_d[:])
            for c in range(NC):
                xchunk(c, 0)
            nc.gpsimd.dma_start(wv_sb[:], wv_d[:])
            for c in range(NC):
                xchunk(c, 1)
            umask = cons[:, 0]

            warm = const.tile([P, 1], F32)
            nc.vector.memset(warm[:], 0.0)
            nc.scalar.activation(warm[:], warm[:], EXP)

            kt_r = kv.tile([P, T], FP16)  # K^T [h, t]
            vt_sb = kv.tile([P, T], FP16)  # V^T [h, t]
            vtmp = kv.tile([P, NT, P], FP16)  # XBAR dst: V [t, h] tiles
            vaug = kv.tile([P, NT, NAV], FP16)  # [v | ones]
            nc.vector.memset(vaug[:], 1.0)
            e_all = e_pool.tile([P, e_width], FP16)

            with (
                tc.tile_pool(name="psproj", bufs=4, space="PSUM") as psp,
                tc.tile_pool(name="pssc", bufs=2, space="PSUM") as pssc,
            ):
                # --- static PSUM plan: 4 proj/AV banks + 4 score banks.
                # Half-1 projection tiles reuse half-0's slots (freed by
                # the casts), and the 4 AV banks reuse the proj slots
                # after the half-1 casts. Warm-ups write into the first
                # kt half-0 bank before its real start=True matmul.
                trash = const.tile([P, CHW], FP16)
                nc.vector.memset(trash[:], 0.0)
                proj_ps = {}
                for h in range(2):
                    for kvi in range(2):
                        proj_ps[(h, kvi)] = [
                            psp.tile(
                                [P, CHW], F32, tag="ps", name=f"pj{h}{kvi}{ch}"
                            )
                            for ch in range(2)
                        ]

                for _ in range(NWARM):
                    nc.tensor.matmul(
                        proj_ps[(0, 0)][0][:], trash[:, 0:P], trash[:],
                        start=True, stop=True, skip_group_check=True,
                    )

                def proj_mms(h, kvi, cs):
                    w_src = wk_sb if kvi == 0 else wv_sb
                    for c in cs:
                        for ch in range(2):
                            lo = h * HT + ch * CHW
                            nc.tensor.matmul(
                                proj_ps[(h, kvi)][ch][:],
                                w_src[:, c, :],
                                xt(c)[:, lo : lo + CHW],
                                start=(c == 0),
                                stop=(c == NC - 1),
                            )

                def cast_half(h, kvi, engines):
                    dst = kt_r if kvi == 0 else vt_sb
                    for ch in range(2):
                        lo = h * HT + ch * CHW
                        eng = engines[ch]
                        if eng is nc.scalar:
                            nc.scalar.copy(
                                dst[:, lo : lo + CHW], proj_ps[(h, kvi)][ch][:]
                            )
                        else:
                            nc.vector.tensor_copy(
                                dst[:, lo : lo + CHW], proj_ps[(h, kvi)][ch][:]
                            )

                # --- AV banks: 3 columns per bank, memset + start=False
                # accumulation; allocated progressively as the proj
                # slots they reuse are freed by the casts.
                av_banks = {}

                def av_bank_open(b):
                    av_banks[b] = psp.tile(
                        [P, 512], F32, tag="ps", name=f"avb{b}"
                    )
                    nc.vector.memset(av_banks[b][:], 0.0)

                def av_region(i):
                    b, reg = AV_BANK[i]
                    return av_banks[b][:, REG * reg : REG * reg + NAV]

                # --- scores / exp pieces ------------------------------
                def spiece(j, part):
                    """part 0: A piece (queries [128j, 1024) for j<8,
                    whole row for j>=8); part 1: B piece [1024, 2048)."""
                    if j < 8:
                        wA = 1024 - j * P
                        base, wt = (0, wA) if part == 0 else (wA, 1024)
                    else:
                        base, wt = 0, rw(j)
                    b0 = j * P
                    s_ps = pssc.tile(
                        [P, 1024], F32, tag="ps", name=f"sp{j}_{part}"
                    )
                    p2 = 0
                    while p2 < wt:
                        w2 = min(CHW, wt - p2)
                        nc.tensor.matmul(
                            s_ps[:, p2 : p2 + w2],
                            kt_r[:, b0 : b0 + P],
                            kt_r[:, b0 + base + p2 : b0 + base + p2 + w2],
                            start=True,
                            stop=True,
                        )
                        p2 += w2
                    nc.scalar.activation(
                        e_all[:, offs[j] + base : offs[j] + base + wt],
                        s_ps[:, :wt],
                        EXP,
                        scale=SCALE,
                    )

                def mask_row(j):
                    nc.vector.tensor_mul(
                        e_all[:, offs[j] : offs[j] + P],
                        e_all[:, offs[j] : offs[j] + P],
                        umask[:],
                    )

                def av_update(j, i):
                    eji = e_all[
                        :, offs[j] + (i - j) * P : offs[j] + (i - j + 1) * P
                    ]
                    nc.tensor.matmul(
                        av_region(i),
                        eji,
                        vaug[:, j, :],
                        start=False,
                        stop=(j == i),
                        skip_group_check=True,
                    )

                y_tiles = {}

                def drain_col(i):
                    g = i // 4
                    if i % 4 == 0:
                        y_tiles[g] = ysb_pool.tile(
                            [P, 4, P], F32, tag="ysb", name=f"ysb{g}"
                        )
                    av = av_region(i)
                    recip = rcp_pool.tile(
                        [P, 1], F32, tag="recip", name=f"rcp{i}"
                    )
                    nc.vector.reciprocal(recip[:], av[:, P : P + 1])
                    nc.vector.tensor_scalar_mul(
                        y_tiles[g][:, i % 4, :], av[:, 0:P], recip[:]
                    )

                def y_out(g):
                    y_view = y_d[512 * g : 512 * (g + 1), :].rearrange(
                        "(i p) h -> p i h", p=P
                    )
                    nc.sync.dma_start(y_view, y_tiles[g][:])

                vaug_done = [0]

                def vaug_copy_upto(n):
                    while vaug_done[0] < min(n, NT):
                        j = vaug_done[0]
                        nc.vector.tensor_copy(
                            vaug[:, j, 0:P], vtmp[:, j, :]
                        )
                        vaug_done[0] += 1

                # --- pre-round emission -------------------------------
                proj_mms(0, 0, range(NC))  # K half-0
                cast_half(0, 0, (nc.scalar, nc.vector))  # kt h0
                spiece(0, 0)  # scores row 0 piece A -> first exp

                # --- round loop ---------------------------------------
                for r in range(NR):
                    for i in DRAINS.get(r, ()):
                        drain_col(i)
                    # masks: rows 0-7 at R0-7, rows 8-15 at R16-23
                    if r < 8:
                        mask_row(r)
                    elif 16 <= r < 24:
                        mask_row(r - 8)
                    # scores pieces: A rows 1-7 at R0-6; B row rb at
                    # R7+rb; rows 8-15 at R15-22
                    if r < 7:
                        spiece(r + 1, 0)
                    elif r < 15:
                        spiece(r - 7, 1)
                    elif r < 23:
                        spiece(r - 7, 0)
                    # V half-0 early (weights land with the half-0 DMA)
                    if r == 0:
                        proj_mms(0, 1, range(0, 4))
                    if r == 1:
                        proj_mms(0, 1, range(4, NC))
                        cast_half(0, 1, (nc.vector, nc.vector))
                        nc.sync.dma_start_transpose(
                            vtmp[:, 0:8, :], vt_sb[:, 0:HT]
                        )
                    # K half-1 as its chunks land; casts feed phase B
                    if 1 <= r <= 4:
                        proj_mms(1, 0, range(2 * (r - 1), 2 * r))
                    if r == 4:
                        cast_half(1, 0, (nc.scalar, nc.vector))
                    if r == 5:
                        av_bank_open(0)  # reuses kt_h1 slots (cast @R4)
                        av_bank_open(1)
                    # V half-1 dribbled over R8-13 (only gates AV js>=8,
                    # first used R16) so the B score pieces aren't
                    # starved behind its matmuls
                    if 8 <= r <= 9:
                        proj_mms(1, 1, range(2 * (r - 8), 2 * (r - 7)))
                    if 10 <= r <= 13:
                        proj_mms(1, 1, [r - 6])
                    if r == 13:
                        cast_half(1, 1, (nc.vector, nc.vector))
                        nc.sync.dma_start_transpose(
                            vtmp[:, 8:16, :], vt_sb[:, HT:T]
                        )
                    if r == 14:
                        av_bank_open(2)  # reuse vt_h1 slots (cast @R13)
                        av_bank_open(3)
                    # inter-generation AV bank re-zeros
                    if r == 9:
                        nc.vector.memset(av_banks[0][:], 0.0)
                    if r == 10:
                        nc.vector.memset(av_banks[1][:], 0.0)
                    # vaug tiles: 0-7 after XBAR-A, 8-15 after XBAR-B
                    # (which is only emitted at R13)
                    if r >= 2:
                        vaug_copy_upto(8 if r < 14 else 8 + 8 * (r - 13))
                    for j, i in AV_SCHED.get(r, ()):
                        av_update(j, i)
                    if r in YDMA:
                        y_out(YDMA[r])

    nc.compile()
    return nc


_NC_CACHE = None


def _get_module():
    global _NC_CACHE
    if _NC_CACHE is None:
        _NC_CACHE = build_module()
    return _NC_CACHE


def run(in_maps, trace=False, **kw):
    nc = _get_module()
    return bass_utils.run_bass_kernel_spmd(
        nc, in_maps, core_ids=list(range(NCORES)), trace=trace, **kw
    )


def make_in_maps(x, W_key, W_value):
    x = np.asarray(x, dtype=np.float32).astype(np.float16)
    xT = x.transpose(0, 2, 1)  # [B, C, T]
    # [B, 2, C, HT]: t-halves contiguous
    xT = np.ascontiguousarray(
        xT.reshape(NCORES, C, 2, HT).transpose(0, 2, 1, 3)
    )
    wk = np.asarray(W_key, np.float32).astype(np.float16)
    wk = np.ascontiguousarray(wk.reshape(NC, P, H).transpose(1, 0, 2))
    wv = np.asarray(W_value, np.float32).astype(np.float16)
    wv = np.ascontiguousarray(wv.reshape(NC, P, H).transpose(1, 0, 2))
    umask = np.triu(np.ones((P, P), dtype=np.float16))  # keep q >= k
    ident = np.eye(P, dtype=np.float16)
    cons = np.ascontiguousarray(np.stack([umask, ident], axis=1))
    return [
        {"xT": xT[b], "WK": wk, "WV": wv, "CONS": cons}
        for b in range(NCORES)
    ]


def kernel(x, W_key, W_query, W_value):
    # W_query intentionally unused: the reference applies W_key for q too.
    del W_query
    res = run(make_in_maps(x, W_key, W_value), trace=False)
    return np.stack([res.results[b]["y"] for b in range(NCORES)], axis=0)
